# revision 27
# baseline (speedup 1.0000x reference)
"""Trainium2 Bass kernel for nn_DWTModelFullBand.

The reference computes a 2-level 2D Haar DWT (wavedec2) and immediately
inverts it (waverec2) reusing the cached level-1 detail bands. idwt2 is the
exact algebraic inverse of dwt2 (orthonormal Haar), so the whole pipeline is
the identity map on x; in fp32 the reference output differs from x only by
rounding noise (~6e-8 relative L2), the same magnitude any re-implementation
with different operation order would produce. The memory-roofline kernel is
therefore a pure copy: read x once from HBM, write it once.

Sharding: pure data parallel over batch — B=32 split as 4 samples per core
across 8 NeuronCores; each core DMA-copies its 12.58 MB shard DRAM->DRAM.

DMA schedule: each HWDGE dma_start's descriptors are dealt to SDMA engines
0..rows-1 in order (the deal restarts at engine 0 for every dma_start —
verified from SDMA packet traces). Engine 15 (E79) also hosts the DGE ring
processing and intermittently sustains only ~17 GB/s vs ~21 GB/s for the
other engines; with a uniform spray it straggles ~7-10 us past everyone
else. Each of the two queues (Sync + Scalar) therefore mixes 16-row
dma_starts (all engines get one row) with 15-row dma_starts (E15 skipped),
sized so E15 carries ~0.78x the bytes of the other engines. The 15-row
starts are issued LAST so all of E15's rows are dealt early, where its rate
is reliable — measured: even a slow-mode E15 (17 GB/s) finishes ~1 us before
the 15 fast engines. Element counts are
chosen so bass's AP splitter reproduces the intended row shapes: a 16*r
contiguous chunk splits into 16 rows of r for any r <= 16384, and a 15*r
chunk splits into 15 rows iff 16 does not divide r. Every dma_start carries
.then_inc(sem, 16) — HWDGE requires sync info, and the 16 four-byte sem
packets (one per engine, ~9 ns each) are dealt after that start's data rows,
so the final wait value 16*n_dma implies all data has landed.

Measured on 8 axon trn2 cores: ~49 us vs 56-57.4 us for the uniform-spray
baseline (prologue ~8.7 us is NEFF/NRT fixed cost; data window ~38.5 us at
~330 GB/s aggregate vs the 360 GB/s 16-engine spec; ~1.2 us completion
tail).
"""

import numpy as np

_B, _C, _H, _W = 32, 3, 512, 512
_NCORES = 8
_BS = _B // _NCORES  # batch shard per core
_SHARD_ELEMS = _BS * _C * _H * _W  # 3,145,728 f32 = 12.58 MB

# Per-queue stream: (rows, row_elems) per dma_start, laid out contiguously.
# E15 share = sum of the 16-row r's = 77,994 elems/queue (624 KB total);
# the other 15 engines additionally split the 15-row parts: +21,664 elems
# -> E15/others = 0.78, sized so E15's slow mode (~16.2 GB/s) finishes no
# later than the 15 fast engines (~21 GB/s x 797 KB). 16-row starts first,
# 15-row starts last. Row bytes are capped at 65,536 (descriptor elem_size
# is a 16-bit byte field; 65,536 encodes as 0 = max — >64KB rows fail
# walrus codegen), so 16,384 f32 elems is the widest possible row.
_PARTS = [
    (16, 16384),
    (16, 16384),
    (16, 16384),
    (16, 16384),
    (16, 12458),
    (15, 10831),
    (15, 10833),
]
_QUEUE_ELEMS = sum(r * n for r, n in _PARTS)
assert 2 * _QUEUE_ELEMS == _SHARD_ELEMS
for _r, _n in _PARTS:
    assert _n <= 16384 and (_r == 16 or (_r == 15 and _n % 16 != 0))

_cache = {}


def _build_nc():
    import concourse.bass as bass
    import concourse.mybir as mybir

    nc = bass.Bass()

    # Drop the init const-pool MEMSETs and the post-init all-engine barrier
    # EventSemaphores (keep the Drains: the profiler keys its useful-time
    # window off them). The kernel body reads no SBUF constants and needs no
    # cross-engine ordering at entry.
    blk = nc.m.functions[0].blocks[0]
    blk.instructions = [
        i
        for i in blk.instructions
        if not (
            isinstance(i, mybir.InstMemset)
            or str(getattr(i, "name", "")).startswith("barrier_")
        )
    ]

    x = nc.declare_dram_parameter("x", [_SHARD_ELEMS], mybir.dt.float32, isOutput=False)
    y = nc.declare_dram_parameter("y", [_SHARD_ELEMS], mybir.dt.float32, isOutput=True)

    n_dma = 0
    with nc.semaphore("dma_sem") as dma_sem:
        for qi, eng in enumerate((nc.sync, nc.scalar)):
            o = qi * _QUEUE_ELEMS
            for rows, row_elems in _PARTS:
                p = rows * row_elems
                sl = slice(o, o + p)
                eng.dma_start(out=y[sl], in_=x[sl]).then_inc(dma_sem, 16)
                o += p
                n_dma += 1
        nc.sync.wait_ge(dma_sem, 16 * n_dma)

    return nc


def _get_nc():
    if "nc" not in _cache:
        _cache["nc"] = _build_nc()
    return _cache["nc"]


def kernel(x: np.ndarray, *, _trace: bool = False, _tmpdir: str | None = None) -> np.ndarray:
    from concourse.bass_utils import run_bass_kernel_spmd

    x = np.ascontiguousarray(np.asarray(x), dtype=np.float32)
    assert x.shape == (_B, _C, _H, _W), x.shape

    nc = _get_nc()
    shards = x.reshape(_NCORES, _SHARD_ELEMS)
    in_maps = [{"x": shards[i]} for i in range(_NCORES)]
    res = run_bass_kernel_spmd(
        nc, in_maps, core_ids=list(range(_NCORES)), trace=_trace, tmpdir=_tmpdir
    )
    _cache["last_result"] = res
    out = np.concatenate([r["y"] for r in res.results])
    return out.reshape(_B, _C, _H, _W)


# revision 28
# speedup vs baseline: 1.1285x; 1.1285x over previous
"""Trainium2 Bass kernel for nn_DWTModelFullBand.

The reference computes a 2-level 2D Haar DWT (wavedec2) and immediately
inverts it (waverec2) reusing the cached level-1 detail bands. idwt2 is the
exact algebraic inverse of dwt2 (orthonormal Haar), so the whole pipeline is
the identity map on x; in fp32 the reference output differs from x only by
rounding noise (~6e-8 relative L2), the same magnitude any re-implementation
with different operation order would produce. The memory-roofline kernel is
therefore a pure copy: read x once from HBM, write it once.

Sharding: pure data parallel over batch — B=32 split as 4 samples per core
across 8 NeuronCores; each core DMA-copies its 12.58 MB shard DRAM->DRAM.

DMA schedule: each HWDGE dma_start's descriptors are dealt to SDMA engines
0..rows-1 in order (the deal restarts at engine 0 for every dma_start —
verified from SDMA packet traces). Engine 15 (E79) also hosts the DGE ring
processing and intermittently sustains only ~17 GB/s vs ~21 GB/s for the
other engines; with a uniform spray it straggles ~7-10 us past everyone
else. Each of the two queues (Sync + Scalar) therefore mixes 16-row
dma_starts (all engines get one row) with 15-row dma_starts (E15 skipped),
sized so E15 carries ~0.78x the bytes of the other engines. The 15-row
starts are issued LAST so all of E15's rows are dealt early, where its rate
is reliable — measured: even a slow-mode E15 (17 GB/s) finishes ~1 us before
the 15 fast engines. Element counts are
chosen so bass's AP splitter reproduces the intended row shapes: a 16*r
contiguous chunk splits into 16 rows of r for any r <= 16384, and a 15*r
chunk splits into 15 rows iff 16 does not divide r. Every dma_start carries
.then_inc(sem, 16) — HWDGE requires sync info, and the 16 four-byte sem
packets (one per engine, ~9 ns each) are dealt after that start's data rows,
so the final wait value 16*n_dma implies all data has landed.

Measured on 8 axon trn2 cores: ~49 us vs 56-57.4 us for the uniform-spray
baseline (prologue ~8.7 us is NEFF/NRT fixed cost; data window ~38.5 us at
~330 GB/s aggregate vs the 360 GB/s 16-engine spec; ~1.2 us completion
tail).
"""

import numpy as np

_B, _C, _H, _W = 32, 3, 512, 512
_NCORES = 8
_BS = _B // _NCORES  # batch shard per core
_SHARD_ELEMS = _BS * _C * _H * _W  # 3,145,728 f32 = 12.58 MB

# Per-queue stream: (rows, row_elems) per dma_start, laid out contiguously.
# E15 share = sum of the 16-row r's = 77,994 elems/queue (624 KB total);
# the other 15 engines additionally split the 15-row parts: +21,664 elems
# -> E15/others = 0.78, sized so E15's slow mode (~16.2 GB/s) finishes no
# later than the 15 fast engines (~21 GB/s x 797 KB). 16-row starts first,
# 15-row starts last. Row bytes are capped at 65,536 (descriptor elem_size
# is a 16-bit byte field; 65,536 encodes as 0 = max — >64KB rows fail
# walrus codegen), so 16,384 f32 elems is the widest possible row.
_PARTS = [
    (16, 16384),
    (16, 16384),
    (16, 16384),
    (16, 16384),
    (16, 12458),
    (15, 10831),
    (15, 10833),
]
_QUEUE_ELEMS = sum(r * n for r, n in _PARTS)
assert 2 * _QUEUE_ELEMS == _SHARD_ELEMS
for _r, _n in _PARTS:
    assert _n <= 16384 and (_r == 16 or (_r == 15 and _n % 16 != 0))

_cache = {}


def _build_nc():
    import concourse.bass as bass
    import concourse.mybir as mybir

    nc = bass.Bass()
    x = nc.declare_dram_parameter("x", [_SHARD_ELEMS], mybir.dt.float32, isOutput=False)
    y = nc.declare_dram_parameter("y", [_SHARD_ELEMS], mybir.dt.float32, isOutput=True)

    n_dma = 0
    with nc.semaphore("dma_sem") as dma_sem:
        for qi, eng in enumerate((nc.sync, nc.scalar)):
            o = qi * _QUEUE_ELEMS
            for rows, row_elems in _PARTS:
                p = rows * row_elems
                sl = slice(o, o + p)
                eng.dma_start(out=y[sl], in_=x[sl]).then_inc(dma_sem, 16)
                o += p
                n_dma += 1
        nc.sync.wait_ge(dma_sem, 16 * n_dma)

    return nc


def _get_nc():
    if "nc" not in _cache:
        _cache["nc"] = _build_nc()
    return _cache["nc"]


def kernel(x: np.ndarray, *, _trace: bool = False, _tmpdir: str | None = None) -> np.ndarray:
    from concourse.bass_utils import run_bass_kernel_spmd

    x = np.ascontiguousarray(np.asarray(x), dtype=np.float32)
    assert x.shape == (_B, _C, _H, _W), x.shape

    nc = _get_nc()
    shards = x.reshape(_NCORES, _SHARD_ELEMS)
    in_maps = [{"x": shards[i]} for i in range(_NCORES)]
    res = run_bass_kernel_spmd(
        nc, in_maps, core_ids=list(range(_NCORES)), trace=_trace, tmpdir=_tmpdir
    )
    _cache["last_result"] = res
    out = np.concatenate([r["y"] for r in res.results])
    return out.reshape(_B, _C, _H, _W)
